# revision 6
# baseline (speedup 1.0000x reference)
"""Trainium2 Bass kernel for nn_Decoder (GRU decoder with batch-contracting
attention), SPMD over 8 NeuronCores, data-parallel over batch.

Math (per reference):
  h0 = fcn_latent                                  (bz, 64)
  step0: attend(h0, enc[:,0]) -> att0 -> GRU1(att0, tile(h0,4)) -> h (bz,256)
  steps 1..99: attend(h_prev, enc[:,t]) -> att -> GRU2 -> h
  out[:, t, :] = h_t @ Wout.T + bout

Attention contracts over the batch: scores = h^T @ enc_t (256x64 global sum),
so each core computes a partial scores matrix over its 512-row batch shard and
the partials are summed with an AllReduce (64KB) each step; everything else is
batch-local.

On-chip layout (per core, batch shard B=512):
  hT    [128, 2*512]  feature-major h^T   (k-half m at cols m*512..)
  h_bm  [128, 4*256]  batch-major h       (col = bt*256 + k)
  enc_bm tile [128, 4, 64]  enc[:, t, :] batch-major (partition=b%128, bt, d)
  encT  [64, 512]     feature-major enc_t
Weights are pre-transposed on the host into lhsT-friendly layouts.

The wide (N=512) matmuls run in float32r (1 cyc/row vs fp32's 4); walrus
requires every operand of an f32r matmul to be *written* as f32r by its
producer, so the activation tiles feeding them are declared float32r and
non-PE consumers read them via bitcast(f32). Small matmuls (N=64) and PE
transposes stay fp32.
"""

import sys

if "/opt/trn_rl_repo" not in sys.path:
    sys.path.insert(0, "/opt/trn_rl_repo")

import numpy as np

BZ, NS, DIN, DH, DOUT = 4096, 100, 64, 256, 64
NCORES = 8
B = BZ // NCORES  # 512 local batch
NBT = B // 128    # 4 batch tiles

_CACHE = {}


def _build(n_steps, mm_f32r=True, enable_asserts=False):
    """Build + compile the Bacc program. Returns nc."""
    import concourse.bass as bass
    import concourse.bacc as bacc
    import concourse.tile as tile
    import concourse.mybir as mybir
    from concourse import masks

    dt = mybir.dt
    f32 = dt.float32
    mmdt = dt.float32r if mm_f32r else f32
    AF = mybir.ActivationFunctionType
    ALU = mybir.AluOpType

    def asf32(ap):  # read an mmdt tile as plain f32
        return ap.bitcast(f32) if mm_f32r else ap

    nc = bacc.Bacc("TRN2", target_bir_lowering=False, debug=False,
                   enable_asserts=enable_asserts, num_devices=NCORES)

    # ---- DRAM I/O (per-core shapes) ----
    h0_d = nc.dram_tensor("h0", [B, DIN], f32, kind="ExternalInput")
    enc_d = nc.dram_tensor("enc", [B, NS, DIN], f32, kind="ExternalInput")
    wa1t_d = nc.dram_tensor("wa1t", [2 * DIN, DIN], f32, kind="ExternalInput")
    ba1_d = nc.dram_tensor("ba1c", [DIN, 1], f32, kind="ExternalInput")
    wa2t_d = nc.dram_tensor("wa2t", [2 * DH, DH], f32, kind="ExternalInput")
    ba2_d = nc.dram_tensor("ba2c", [DH, 1], f32, kind="ExternalInput")
    wih1t_d = nc.dram_tensor("wih1t", [DIN, 3 * DH], f32, kind="ExternalInput")
    whh1et_d = nc.dram_tensor("whh1et", [DIN, 3 * DH], f32, kind="ExternalInput")
    b1rz_d = nc.dram_tensor("b1rz", [2 * DH, 1], f32, kind="ExternalInput")
    b1in_d = nc.dram_tensor("b1in", [DH, 1], f32, kind="ExternalInput")
    b1hn_d = nc.dram_tensor("b1hn", [DH, 1], f32, kind="ExternalInput")
    wih2t_d = nc.dram_tensor("wih2t", [DH, 3 * DH], f32, kind="ExternalInput")
    whh2t_d = nc.dram_tensor("whh2t", [DH, 3 * DH], f32, kind="ExternalInput")
    b2rz_d = nc.dram_tensor("b2rz", [2 * DH, 1], f32, kind="ExternalInput")
    b2in_d = nc.dram_tensor("b2in", [DH, 1], f32, kind="ExternalInput")
    b2hn_d = nc.dram_tensor("b2hn", [DH, 1], f32, kind="ExternalInput")
    woutt_d = nc.dram_tensor("woutt", [DH, DOUT], f32, kind="ExternalInput")
    boutr_d = nc.dram_tensor("boutr", [1, NBT * DOUT], f32, kind="ExternalInput")
    out_d = nc.dram_tensor("out", [B, NS, DOUT], f32, kind="ExternalOutput")

    with tile.TileContext(nc) as tc:
        import contextlib
        ctx = contextlib.ExitStack()
        with ctx:
            cpool = ctx.enter_context(tc.tile_pool(name="const", bufs=1))
            spool = ctx.enter_context(tc.tile_pool(name="state", bufs=2))
            wpool = ctx.enter_context(tc.tile_pool(name="work", bufs=2))
            epool = ctx.enter_context(tc.tile_pool(name="encp", bufs=3))
            opool = ctx.enter_context(tc.tile_pool(name="outp", bufs=3))
            prz = ctx.enter_context(tc.tile_pool(name="psum_rz", bufs=4, space="PSUM"))
            pghn = ctx.enter_context(tc.tile_pool(name="psum_ghn", bufs=2, space="PSUM"))
            pmm = ctx.enter_context(tc.tile_pool(name="psum_mm", bufs=2, space="PSUM"))
            ccpool = ctx.enter_context(tc.tile_pool(name="cc", bufs=2, space="DRAM"))

            # ---- constants ----
            ident = cpool.tile([128, 128], f32, tag="ident")
            masks.make_identity(nc, ident[:])

            def load_weight(name, dram_ap, shape, round_to_mm=True):
                """DMA f32 weight, then round-copy into an mmdt tile."""
                raw = cpool.tile(shape, f32, tag=name + "_raw")
                nc.sync.dma_start(raw[:], dram_ap)
                if not mm_f32r or not round_to_mm:
                    return raw
                rnd = cpool.tile(shape, mmdt, tag=name)
                nc.vector.tensor_copy(rnd[:], raw[:])
                return rnd

            wa1t = load_weight("wa1t", wa1t_d.ap(), [128, DIN])
            wa2t = load_weight(
                "wa2t",
                wa2t_d.ap().rearrange("(j p) n -> p j n", p=128),
                [128, 4, DH])
            wih1t = load_weight("wih1t", wih1t_d.ap(), [DIN, 3 * DH])
            whh1et = load_weight("whh1et", whh1et_d.ap(), [DIN, 3 * DH])
            wih2t = load_weight(
                "wih2t",
                wih2t_d.ap().rearrange("(j p) n -> p j n", p=128),
                [128, 2, 3 * DH])
            whh2t = load_weight(
                "whh2t",
                whh2t_d.ap().rearrange("(j p) n -> p j n", p=128),
                [128, 2, 3 * DH])
            woutt = load_weight(
                "woutt",
                woutt_d.ap().rearrange("(j p) n -> p j n", p=128),
                [128, 2, DOUT], round_to_mm=False)

            ba1 = cpool.tile([DIN, 1], f32, tag="ba1")
            nc.sync.dma_start(ba1[:], ba1_d.ap())

            def load_bias(name, dram, j):
                t = cpool.tile([128, j, 1], f32, tag=name)
                nc.sync.dma_start(
                    t[:], dram.ap().rearrange("(j p) o -> p j o", p=128))
                return t

            ba2 = load_bias("ba2", ba2_d, 2)
            b1rz = load_bias("b1rz", b1rz_d, 4)
            b1in = load_bias("b1in", b1in_d, 2)
            b1hn = load_bias("b1hn", b1hn_d, 2)
            b2rz = load_bias("b2rz", b2rz_d, 4)
            b2in = load_bias("b2in", b2in_d, 2)
            b2hn = load_bias("b2hn", b2hn_d, 2)

            boutr_row = cpool.tile([1, NBT * DOUT], f32, tag="boutr_row")
            nc.sync.dma_start(boutr_row[:], boutr_d.ap())
            boutb = cpool.tile([128, NBT * DOUT], f32, tag="boutb")
            nc.gpsimd.partition_broadcast(boutb[:], boutr_row[:])

            enc_r = enc_d.ap().rearrange("(bt p) t d -> p bt t d", p=128)
            out_r = out_d.ap().rearrange("(bt p) t d -> p bt t d", p=128)

            def load_enc(t):
                e = epool.tile([128, NBT, DIN], f32, tag="enc")
                nc.sync.dma_start(e[:], enc_r[:, :, t, :])
                return e

            def make_encT(e_bm):
                """[128, NBT, 64] batch-major -> [64, 512] feature-major."""
                pt = pmm.tile([DIN, B], f32, tag="mm")
                for bt in range(NBT):
                    nc.tensor.transpose(
                        pt[:, bt * 128:(bt + 1) * 128], e_bm[:, bt, :], ident[:])
                eT = wpool.tile([DIN, B], mmdt, tag="encT")
                nc.scalar.copy(eT[:], pt[:])
                return eT

            def softmax_weights(sc_sb, parts, nd):
                """sc_sb [parts, nd*64]: per 64-col group softmax (unnormalized
                exp + reciprocal-of-sum). Returns (w_sb, rinv)."""
                negmax = wpool.tile([parts, nd], f32, tag="negmax")
                ssum = wpool.tile([parts, nd], f32, tag="ssum")
                w_sb = wpool.tile([parts, nd * DIN], f32, tag="w_sb")
                for m in range(nd):
                    nc.vector.reduce_max(
                        negmax[:, m:m + 1], sc_sb[:, m * DIN:(m + 1) * DIN],
                        axis=mybir.AxisListType.X, negate=True)
                    nc.scalar.activation(
                        w_sb[:, m * DIN:(m + 1) * DIN],
                        sc_sb[:, m * DIN:(m + 1) * DIN],
                        AF.Exp, bias=negmax[:, m:m + 1],
                        accum_out=ssum[:, m:m + 1])
                rinv = wpool.tile([parts, nd], f32, tag="rinv")
                nc.vector.reciprocal(rinv[:], ssum[:])
                return w_sb, rinv

            def allreduce(sb_tile, parts, cols):
                """DMA sb_tile -> DRAM, AllReduce, DMA back. Returns sbuf tile."""
                cin = ccpool.tile([parts, cols], f32, tag="cin")
                cout = ccpool.tile([parts, cols], f32, tag="cout")
                nc.sync.dma_start(cin[:], sb_tile[:])
                nc.gpsimd.collective_compute(
                    "AllReduce", ALU.add,
                    replica_groups=[list(range(NCORES))],
                    ins=[cin.opt()], outs=[cout.opt()])
                red = wpool.tile([parts, cols], f32, tag="scred")
                nc.sync.dma_start(red[:], cout[:])
                return red

            def outproj(hT, t):
                """out[:, t, :] = h @ Wout.T + bout (batch-major store)."""
                po = pmm.tile([128, NBT * DOUT], f32, tag="mm")
                for bt in range(NBT):
                    for j in range(2):
                        nc.tensor.matmul(
                            po[:, bt * DOUT:(bt + 1) * DOUT],
                            asf32(hT[:, j * B + bt * 128: j * B + (bt + 1) * 128]),
                            woutt[:, j, :],
                            start=(j == 0), stop=(j == 1))
                ob = opool.tile([128, NBT, DOUT], f32, tag="ob")
                nc.vector.tensor_add(
                    ob[:].rearrange("p bt d -> p (bt d)"), po[:], boutb[:])
                nc.sync.dma_start(out_r[:, :, t, :], ob[:])

            def make_hbm(hT):
                """hT [128, 2*512] -> h_bm [128, 4*256] (col = bt*256 + k)."""
                hbm = spool.tile([128, NBT * DH], f32, tag="hbm")
                for m in range(2):
                    pt = pmm.tile([128, B], f32, tag="mm")
                    for bt in range(NBT):
                        nc.tensor.transpose(
                            pt[:, bt * 128:(bt + 1) * 128],
                            asf32(hT[:, m * B + bt * 128: m * B + (bt + 1) * 128]),
                            ident[:])
                    nc.scalar.copy(
                        hbm[:].rearrange("p (bt k) -> p bt k", bt=NBT)
                           [:, :, m * 128:(m + 1) * 128],
                        pt[:].rearrange("p (bt k) -> p bt k", bt=NBT))
                return hbm

            def gru_elementwise(p_rz, p_gin, p_ghn, hT_old_halves, brz, bin_, bhn):
                """Generic GRU gate math on feature-major halves.
                p_rz: list of 4 psum tiles [128, B] (r0, r1, z0, z1)
                p_gin/p_ghn: lists of 2 psum tiles [128, B]
                hT_old_halves: list of 2 SBUF f32 APs [128, B]
                Returns new hT tile [128, 2*B] (mmdt)."""
                hT_new = spool.tile([128, 2 * B], mmdt, tag="hT")
                r = wpool.tile([128, 2 * B], f32, tag="r")
                z = wpool.tile([128, 2 * B], f32, tag="z")
                n = wpool.tile([128, 2 * B], f32, tag="n")
                for m in range(2):
                    sl = slice(m * B, (m + 1) * B)
                    nc.scalar.activation(r[:, sl], p_rz[m][:], AF.Sigmoid,
                                         bias=brz[:, m, :])
                    nc.scalar.activation(z[:, sl], p_rz[2 + m][:], AF.Sigmoid,
                                         bias=brz[:, 2 + m, :])
                    tmp = wpool.tile([128, B], f32, tag="tmp")
                    # tmp = (ghn + bhn) * r
                    nc.vector.scalar_tensor_tensor(
                        tmp[:], p_ghn[m][:], bhn[:, m, :], r[:, sl],
                        op0=ALU.add, op1=ALU.mult)
                    c = wpool.tile([128, B], f32, tag="c")
                    # c = (gin + bin) + tmp
                    nc.vector.scalar_tensor_tensor(
                        c[:], p_gin[m][:], bin_[:, m, :], tmp[:],
                        op0=ALU.add, op1=ALU.add)
                    nc.scalar.activation(n[:, sl], c[:], AF.Tanh)
                    hmn = wpool.tile([128, B], f32, tag="hmn")
                    nc.vector.tensor_sub(hmn[:], hT_old_halves[m], n[:, sl])
                    e = wpool.tile([128, B], f32, tag="e")
                    nc.vector.tensor_mul(e[:], z[:, sl], hmn[:])
                    nc.vector.tensor_add(hT_new[:, sl], n[:, sl], e[:])
                return hT_new

            # ================= step 0 =================
            h0bm = spool.tile([128, NBT, DIN], f32, tag="h0bm")
            nc.sync.dma_start(
                h0bm[:], h0_d.ap().rearrange("(bt p) d -> p bt d", p=128))
            e0 = load_enc(0)

            # scores0 partial [64, 64] = sum_bt h0bm[bt].T @ e0[bt]
            psc0 = pmm.tile([DIN, DIN], f32, tag="mm")
            for bt in range(NBT):
                nc.tensor.matmul(psc0[:], h0bm[:, bt, :], e0[:, bt, :],
                                 start=(bt == 0), stop=(bt == NBT - 1))
            sc0 = wpool.tile([DIN, DIN], f32, tag="sc0")
            nc.vector.tensor_copy(sc0[:], psc0[:])
            red0 = allreduce(sc0, DIN, DIN)
            w0, rinv0 = softmax_weights(red0, DIN, 1)

            # w0T [64, 64]
            pw0 = pmm.tile([DIN, DIN], f32, tag="mm")
            nc.tensor.transpose(pw0[:], w0[:], ident[0:DIN, 0:DIN])
            w0t = wpool.tile([DIN, DIN], mmdt, tag="w0t")
            nc.vector.tensor_copy(w0t[:], pw0[:])

            e0T = make_encT(e0)

            # h0T [64, 512] (psum)
            ph0 = pmm.tile([DIN, B], f32, tag="mm")
            for bt in range(NBT):
                nc.tensor.transpose(ph0[:, bt * 128:(bt + 1) * 128],
                                    h0bm[:, bt, :], ident[:])
            # cat0 [128, 512]: rows 0:64 = ctx0T (scaled), rows 64:128 = h0T
            cat0 = wpool.tile([128, B], mmdt, tag="cat0")
            nc.scalar.copy(cat0[DIN:128, :], ph0[:])

            pctx0 = pmm.tile([DIN, B], f32, tag="mm")
            nc.tensor.matmul(pctx0[:], w0t[:], e0T[:], start=True, stop=True)
            nc.vector.tensor_scalar_mul(cat0[0:DIN, :], pctx0[:],
                                        rinv0[:, 0:1])

            # att0T [64, 512] = tanh(Wa1T.T @ cat0 + ba1)
            patt0 = pmm.tile([DIN, B], f32, tag="mm")
            nc.tensor.matmul(patt0[:], wa1t[:], cat0[:], start=True, stop=True)
            att0T = wpool.tile([DIN, B], mmdt, tag="att0T")
            nc.scalar.activation(att0T[:], patt0[:], AF.Tanh, bias=ba1[:])

            # hrep [128, 512] = [h0T; h0T]  (both halves from psum ph0)
            hrep = wpool.tile([128, B], mmdt, tag="hrep")
            nc.scalar.copy(hrep[0:DIN, :], ph0[:])
            nc.scalar.copy(hrep[DIN:128, :], ph0[:])

            # GRU1 gates
            p_rz = []
            for m in range(4):
                pg = prz.tile([128, B], f32, tag="rz")
                nc.tensor.matmul(pg[:], wih1t[:, m * 128:(m + 1) * 128],
                                 att0T[:], start=True, stop=False)
                nc.tensor.matmul(pg[:], whh1et[:, m * 128:(m + 1) * 128],
                                 hrep[0:DIN, :], start=False, stop=True)
                p_rz.append(pg)
            p_gin, p_ghn = [], []
            for m in range(2):
                pg = pmm.tile([128, B], f32, tag="mm")
                nc.tensor.matmul(
                    pg[:], wih1t[:, 512 + m * 128: 512 + (m + 1) * 128],
                    att0T[:], start=True, stop=True)
                p_gin.append(pg)
            for m in range(2):
                pg = pghn.tile([128, B], f32, tag="ghn")
                nc.tensor.matmul(
                    pg[:], whh1et[:, 512 + m * 128: 512 + (m + 1) * 128],
                    hrep[0:DIN, :], start=True, stop=True)
                p_ghn.append(pg)

            hrep_f32 = asf32(hrep[:])
            hT = gru_elementwise(p_rz, p_gin, p_ghn,
                                 [hrep_f32, hrep_f32], b1rz, b1in, b1hn)
            outproj(hT, 0)
            hbm = make_hbm(hT)

            # ================= steps 1..n_steps-1 =================
            for t in range(1, n_steps):
                e_bm = load_enc(t)

                # partial scores [128, 2*64]: m-tile m at cols m*64..
                psc = pmm.tile([128, 2 * DIN], f32, tag="mm")
                for m in range(2):
                    for bt in range(NBT):
                        nc.tensor.matmul(
                            psc[:, m * DIN:(m + 1) * DIN],
                            hbm[:].rearrange("p (bt k) -> p bt k", bt=NBT)
                               [:, bt, m * 128:(m + 1) * 128],
                            e_bm[:, bt, :],
                            start=(bt == 0), stop=(bt == NBT - 1))
                sc = wpool.tile([128, 2 * DIN], f32, tag="sc")
                nc.vector.tensor_copy(sc[:], psc[:])

                red = allreduce(sc, 128, 2 * DIN)

                # hT-only matmuls fill the AllReduce window: ghn + the Whh
                # half of the r/z gate accumulations (start the psum groups).
                p_ghn = []
                for m in range(2):
                    pg = pghn.tile([128, B], f32, tag="ghn")
                    for j in range(2):
                        nc.tensor.matmul(
                            pg[:], whh2t[:, j, 512 + m * 128: 512 + (m + 1) * 128],
                            hT[:, j * B:(j + 1) * B],
                            start=(j == 0), stop=(j == 1))
                    p_ghn.append(pg)
                p_rz = []
                for m in range(4):
                    pg = prz.tile([128, B], f32, tag="rz")
                    for j in range(2):
                        nc.tensor.matmul(
                            pg[:], whh2t[:, j, m * 128:(m + 1) * 128],
                            hT[:, j * B:(j + 1) * B],
                            start=(j == 0), stop=False)
                    p_rz.append(pg)

                w_sb, rinv = softmax_weights(red, 128, 2)

                # wT [64, 256]
                pw = pmm.tile([DIN, DH], f32, tag="mm")
                for m in range(2):
                    nc.tensor.transpose(pw[:, m * 128:(m + 1) * 128],
                                        w_sb[:, m * DIN:(m + 1) * DIN],
                                        ident[:])
                wt = wpool.tile([DIN, DH], mmdt, tag="wt")
                nc.vector.tensor_copy(wt[:], pw[:])

                eT = make_encT(e_bm)

                # ctxT [128, 2*512] scaled by rinv
                ctxT = wpool.tile([128, 2 * B], mmdt, tag="ctxT")
                for m in range(2):
                    pc = pmm.tile([128, B], f32, tag="mm")
                    nc.tensor.matmul(pc[:], wt[:, m * 128:(m + 1) * 128],
                                     eT[:], start=True, stop=True)
                    nc.vector.tensor_scalar_mul(
                        ctxT[:, m * B:(m + 1) * B], pc[:], rinv[:, m:m + 1])

                # attT = tanh(Wa2T.T @ [ctxT; hT] + ba2)
                attT = wpool.tile([128, 2 * B], mmdt, tag="attT")
                for m in range(2):
                    pa = pmm.tile([128, B], f32, tag="mm")
                    for j in range(4):
                        rhs = (ctxT[:, j * B:(j + 1) * B] if j < 2
                               else hT[:, (j - 2) * B:(j - 1) * B])
                        nc.tensor.matmul(
                            pa[:], wa2t[:, j, m * 128:(m + 1) * 128],
                            rhs, start=(j == 0), stop=(j == 3))
                    nc.scalar.activation(attT[:, m * B:(m + 1) * B], pa[:],
                                         AF.Tanh, bias=ba2[:, m, :])

                # finish r/z gates with the Wih half (att-dependent)
                for m in range(4):
                    pg = p_rz[m]
                    for j in range(2):
                        nc.tensor.matmul(
                            pg[:], wih2t[:, j, m * 128:(m + 1) * 128],
                            attT[:, j * B:(j + 1) * B],
                            start=False, stop=(j == 1))
                p_gin = []
                for m in range(2):
                    pg = pmm.tile([128, B], f32, tag="mm")
                    for j in range(2):
                        nc.tensor.matmul(
                            pg[:], wih2t[:, j, 512 + m * 128: 512 + (m + 1) * 128],
                            attT[:, j * B:(j + 1) * B],
                            start=(j == 0), stop=(j == 1))
                    p_gin.append(pg)

                hT_old_halves = [asf32(hT[:, 0:B]), asf32(hT[:, B:2 * B])]
                hT = gru_elementwise(p_rz, p_gin, p_ghn, hT_old_halves,
                                     b2rz, b2in, b2hn)
                outproj(hT, t)
                if t < n_steps - 1:
                    hbm = make_hbm(hT)

    nc.compile()
    return nc


def _get_nc(n_steps=NS, mm_f32r=True):
    key = (n_steps, mm_f32r)
    if key not in _CACHE:
        _CACHE[key] = _build(n_steps, mm_f32r=mm_f32r)
    return _CACHE[key]


def _prep_weights(inputs):
    g = {k: np.ascontiguousarray(np.asarray(v, dtype=np.float32))
         for k, v in inputs.items()}
    Whh1 = g["Whh1"]
    whh1e = Whh1[:, 0:64] + Whh1[:, 64:128] + Whh1[:, 128:192] + Whh1[:, 192:256]
    wd = {
        "wa1t": g["Wa1"].T,
        "ba1c": g["ba1"][:, None],
        "wa2t": g["Wa2"].T,
        "ba2c": g["ba2"][:, None],
        "wih1t": g["Wih1"].T,
        "whh1et": whh1e.T,
        "b1rz": (g["bih1"] + g["bhh1"])[:512, None],
        "b1in": g["bih1"][512:, None],
        "b1hn": g["bhh1"][512:, None],
        "wih2t": g["Wih2"].T,
        "whh2t": g["Whh2"].T,
        "b2rz": (g["bih2"] + g["bhh2"])[:512, None],
        "b2in": g["bih2"][512:, None],
        "b2hn": g["bhh2"][512:, None],
        "woutt": g["Wout"].T,
        "boutr": np.tile(g["bout"], NBT)[None, :],
    }
    return {k: np.ascontiguousarray(v, dtype=np.float32) for k, v in wd.items()}


def run(inputs, n_steps=NS, mm_f32r=True, trace=False):
    from concourse import bass_utils
    nc = _get_nc(n_steps, mm_f32r)
    fcn = np.ascontiguousarray(np.asarray(inputs["fcn_latent"], np.float32))
    enc = np.ascontiguousarray(np.asarray(inputs["encoder_outputs"], np.float32))
    wd = _prep_weights(inputs)
    in_maps = []
    for c in range(NCORES):
        m = dict(wd)
        m["h0"] = np.ascontiguousarray(fcn[c * B:(c + 1) * B])
        m["enc"] = np.ascontiguousarray(enc[c * B:(c + 1) * B])
        in_maps.append(m)
    res = bass_utils.run_bass_kernel_spmd(
        nc, in_maps, core_ids=list(range(NCORES)), trace=trace)
    out = np.concatenate([res.results[c]["out"] for c in range(NCORES)], axis=0)
    return out[:, :n_steps, :], res


def kernel(**inputs) -> np.ndarray:
    out, _ = run(inputs, n_steps=NS, mm_f32r=True)
    return out


# revision 7
# speedup vs baseline: 1.0304x; 1.0304x over previous
"""Trainium2 Bass kernel for nn_Decoder (GRU decoder with batch-contracting
attention), SPMD over 8 NeuronCores, data-parallel over batch.

Math (per reference):
  h0 = fcn_latent                                  (bz, 64)
  step0: attend(h0, enc[:,0]) -> att0 -> GRU1(att0, tile(h0,4)) -> h (bz,256)
  steps 1..99: attend(h_prev, enc[:,t]) -> att -> GRU2 -> h
  out[:, t, :] = h_t @ Wout.T + bout

Attention contracts over the batch: scores = h^T @ enc_t (256x64 global sum),
so each core computes a partial scores matrix over its 512-row batch shard and
the partials are summed with an AllReduce (64KB) each step; everything else is
batch-local.

On-chip layout (per core, batch shard B=512):
  hT    [128, 2*512]  feature-major h^T   (k-half m at cols m*512..)
  h_bm  [128, 4*256]  batch-major h       (col = bt*256 + k)
  enc_bm tile [128, 4, 64]  enc[:, t, :] batch-major (partition=b%128, bt, d)
  encT  [64, 512]     feature-major enc_t
Weights are pre-transposed on the host into lhsT-friendly layouts.

The wide (N=512) matmuls run in float32r (1 cyc/row vs fp32's 4); walrus
requires every operand of an f32r matmul to be *written* as f32r by its
producer, so the activation tiles feeding them are declared float32r and
non-PE consumers read them via bitcast(f32). Small matmuls (N=64) and PE
transposes stay fp32.
"""

import sys

if "/opt/trn_rl_repo" not in sys.path:
    sys.path.insert(0, "/opt/trn_rl_repo")

import numpy as np

BZ, NS, DIN, DH, DOUT = 4096, 100, 64, 256, 64
NCORES = 8
B = BZ // NCORES  # 512 local batch
NBT = B // 128    # 4 batch tiles

_CACHE = {}


def _build(n_steps, mm_f32r=True, enable_asserts=False):
    """Build + compile the Bacc program. Returns nc."""
    import concourse.bass as bass
    import concourse.bacc as bacc
    import concourse.tile as tile
    import concourse.mybir as mybir
    from concourse import masks

    dt = mybir.dt
    f32 = dt.float32
    mmdt = dt.float32r if mm_f32r else f32
    AF = mybir.ActivationFunctionType
    ALU = mybir.AluOpType

    def asf32(ap):  # read an mmdt tile as plain f32
        return ap.bitcast(f32) if mm_f32r else ap

    nc = bacc.Bacc("TRN2", target_bir_lowering=False, debug=False,
                   enable_asserts=enable_asserts, num_devices=NCORES)

    # ---- DRAM I/O (per-core shapes) ----
    h0_d = nc.dram_tensor("h0", [B, DIN], f32, kind="ExternalInput")
    enc_d = nc.dram_tensor("enc", [B, NS, DIN], f32, kind="ExternalInput")
    wa1t_d = nc.dram_tensor("wa1t", [2 * DIN, DIN], f32, kind="ExternalInput")
    ba1_d = nc.dram_tensor("ba1c", [DIN, 1], f32, kind="ExternalInput")
    wa2t_d = nc.dram_tensor("wa2t", [2 * DH, DH], f32, kind="ExternalInput")
    ba2_d = nc.dram_tensor("ba2c", [DH, 1], f32, kind="ExternalInput")
    wih1t_d = nc.dram_tensor("wih1t", [DIN, 3 * DH], f32, kind="ExternalInput")
    whh1et_d = nc.dram_tensor("whh1et", [DIN, 3 * DH], f32, kind="ExternalInput")
    b1rz_d = nc.dram_tensor("b1rz", [2 * DH, 1], f32, kind="ExternalInput")
    b1in_d = nc.dram_tensor("b1in", [DH, 1], f32, kind="ExternalInput")
    b1hn_d = nc.dram_tensor("b1hn", [DH, 1], f32, kind="ExternalInput")
    wih2t_d = nc.dram_tensor("wih2t", [DH, 3 * DH], f32, kind="ExternalInput")
    whh2t_d = nc.dram_tensor("whh2t", [DH, 3 * DH], f32, kind="ExternalInput")
    b2rz_d = nc.dram_tensor("b2rz", [2 * DH, 1], f32, kind="ExternalInput")
    b2in_d = nc.dram_tensor("b2in", [DH, 1], f32, kind="ExternalInput")
    b2hn_d = nc.dram_tensor("b2hn", [DH, 1], f32, kind="ExternalInput")
    woutt_d = nc.dram_tensor("woutt", [DH, DOUT], f32, kind="ExternalInput")
    boutr_d = nc.dram_tensor("boutr", [1, NBT * DOUT], f32, kind="ExternalInput")
    out_d = nc.dram_tensor("out", [B, NS, DOUT], f32, kind="ExternalOutput")

    with tile.TileContext(nc) as tc:
        import contextlib
        ctx = contextlib.ExitStack()
        with ctx:
            cpool = ctx.enter_context(tc.tile_pool(name="const", bufs=1))
            spool = ctx.enter_context(tc.tile_pool(name="state", bufs=2))
            wpool = ctx.enter_context(tc.tile_pool(name="work", bufs=2))
            epool = ctx.enter_context(tc.tile_pool(name="encp", bufs=3))
            opool = ctx.enter_context(tc.tile_pool(name="outp", bufs=3))
            prz = ctx.enter_context(tc.tile_pool(name="psum_rz", bufs=4, space="PSUM"))
            pghn = ctx.enter_context(tc.tile_pool(name="psum_ghn", bufs=2, space="PSUM"))
            pmm = ctx.enter_context(tc.tile_pool(name="psum_mm", bufs=2, space="PSUM"))
            ccpool = ctx.enter_context(tc.tile_pool(name="cc", bufs=2, space="DRAM"))

            # ---- constants ----
            ident = cpool.tile([128, 128], f32, tag="ident")
            masks.make_identity(nc, ident[:])

            def load_weight(name, dram_ap, shape, round_to_mm=True):
                """DMA f32 weight, then round-copy into an mmdt tile."""
                raw = cpool.tile(shape, f32, tag=name + "_raw")
                nc.sync.dma_start(raw[:], dram_ap)
                if not mm_f32r or not round_to_mm:
                    return raw
                rnd = cpool.tile(shape, mmdt, tag=name)
                nc.vector.tensor_copy(rnd[:], raw[:])
                return rnd

            wa1t = load_weight("wa1t", wa1t_d.ap(), [128, DIN])
            wa2t = load_weight(
                "wa2t",
                wa2t_d.ap().rearrange("(j p) n -> p j n", p=128),
                [128, 4, DH])
            wih1t = load_weight("wih1t", wih1t_d.ap(), [DIN, 3 * DH])
            whh1et = load_weight("whh1et", whh1et_d.ap(), [DIN, 3 * DH])
            wih2t = load_weight(
                "wih2t",
                wih2t_d.ap().rearrange("(j p) n -> p j n", p=128),
                [128, 2, 3 * DH])
            whh2t = load_weight(
                "whh2t",
                whh2t_d.ap().rearrange("(j p) n -> p j n", p=128),
                [128, 2, 3 * DH])
            woutt = load_weight(
                "woutt",
                woutt_d.ap().rearrange("(j p) n -> p j n", p=128),
                [128, 2, DOUT], round_to_mm=False)

            ba1 = cpool.tile([DIN, 1], f32, tag="ba1")
            nc.sync.dma_start(ba1[:], ba1_d.ap())

            def load_bias(name, dram, j):
                t = cpool.tile([128, j, 1], f32, tag=name)
                nc.sync.dma_start(
                    t[:], dram.ap().rearrange("(j p) o -> p j o", p=128))
                return t

            ba2 = load_bias("ba2", ba2_d, 2)
            b1rz = load_bias("b1rz", b1rz_d, 4)
            b1in = load_bias("b1in", b1in_d, 2)
            b1hn = load_bias("b1hn", b1hn_d, 2)
            b2rz = load_bias("b2rz", b2rz_d, 4)
            b2in = load_bias("b2in", b2in_d, 2)
            b2hn = load_bias("b2hn", b2hn_d, 2)

            boutr_row = cpool.tile([1, NBT * DOUT], f32, tag="boutr_row")
            nc.sync.dma_start(boutr_row[:], boutr_d.ap())
            boutb = cpool.tile([128, NBT * DOUT], f32, tag="boutb")
            nc.gpsimd.partition_broadcast(boutb[:], boutr_row[:])

            enc_r = enc_d.ap().rearrange("(bt p) t d -> p bt t d", p=128)
            out_r = out_d.ap().rearrange("(bt p) t d -> p bt t d", p=128)

            def load_enc(t):
                e = epool.tile([128, NBT, DIN], f32, tag="enc")
                nc.sync.dma_start(e[:], enc_r[:, :, t, :])
                return e

            def make_encT(e_bm):
                """[128, NBT, 64] batch-major -> [64, 512] feature-major."""
                pt = pmm.tile([DIN, B], f32, tag="mm")
                for bt in range(NBT):
                    nc.tensor.transpose(
                        pt[:, bt * 128:(bt + 1) * 128], e_bm[:, bt, :], ident[:])
                eT = wpool.tile([DIN, B], mmdt, tag="encT")
                nc.scalar.copy(eT[:], pt[:])
                return eT

            def softmax_weights(sc_sb, parts, nd):
                """sc_sb [parts, nd*64]: per 64-col group softmax (unnormalized
                exp + reciprocal-of-sum). Returns (w_sb, rinv)."""
                negmax = wpool.tile([parts, nd], f32, tag="negmax")
                ssum = wpool.tile([parts, nd], f32, tag="ssum")
                w_sb = wpool.tile([parts, nd * DIN], f32, tag="w_sb")
                for m in range(nd):
                    nc.vector.reduce_max(
                        negmax[:, m:m + 1], sc_sb[:, m * DIN:(m + 1) * DIN],
                        axis=mybir.AxisListType.X, negate=True)
                    nc.scalar.activation(
                        w_sb[:, m * DIN:(m + 1) * DIN],
                        sc_sb[:, m * DIN:(m + 1) * DIN],
                        AF.Exp, bias=negmax[:, m:m + 1],
                        accum_out=ssum[:, m:m + 1])
                rinv = wpool.tile([parts, nd], f32, tag="rinv")
                nc.vector.reciprocal(rinv[:], ssum[:])
                return w_sb, rinv

            def allreduce(sb_tile, parts, cols):
                """DMA sb_tile -> DRAM, AllReduce, DMA back. Returns sbuf tile."""
                cin = ccpool.tile([parts, cols], f32, tag="cin")
                cout = ccpool.tile([parts, cols], f32, tag="cout")
                nc.sync.dma_start(cin[:], sb_tile[:])
                nc.gpsimd.collective_compute(
                    "AllReduce", ALU.add,
                    replica_groups=[list(range(NCORES))],
                    ins=[cin.opt()], outs=[cout.opt()])
                red = wpool.tile([parts, cols], f32, tag="scred")
                nc.sync.dma_start(red[:], cout[:])
                return red

            def outproj(hT, t):
                """out[:, t, :] = h @ Wout.T + bout (batch-major store)."""
                po = pmm.tile([128, NBT * DOUT], f32, tag="mm")
                for bt in range(NBT):
                    for j in range(2):
                        nc.tensor.matmul(
                            po[:, bt * DOUT:(bt + 1) * DOUT],
                            asf32(hT[:, j * B + bt * 128: j * B + (bt + 1) * 128]),
                            woutt[:, j, :],
                            start=(j == 0), stop=(j == 1))
                ob = opool.tile([128, NBT, DOUT], f32, tag="ob")
                nc.vector.tensor_add(
                    ob[:].rearrange("p bt d -> p (bt d)"), po[:], boutb[:])
                nc.sync.dma_start(out_r[:, :, t, :], ob[:])

            def make_hbm(hT):
                """hT [128, 2*512] -> h_bm [128, 4*256] (col = bt*256 + k)."""
                hbm = spool.tile([128, NBT * DH], f32, tag="hbm")
                for m in range(2):
                    pt = pmm.tile([128, B], f32, tag="mm")
                    for bt in range(NBT):
                        nc.tensor.transpose(
                            pt[:, bt * 128:(bt + 1) * 128],
                            asf32(hT[:, m * B + bt * 128: m * B + (bt + 1) * 128]),
                            ident[:])
                    nc.scalar.copy(
                        hbm[:].rearrange("p (bt k) -> p bt k", bt=NBT)
                           [:, :, m * 128:(m + 1) * 128],
                        pt[:].rearrange("p (bt k) -> p bt k", bt=NBT))
                return hbm

            def gru_elementwise(p_rz, p_gin, p_ghn, hT_old_halves, brz, bin_, bhn):
                """Generic GRU gate math on feature-major halves.
                p_rz: list of 4 psum tiles [128, B] (r0, r1, z0, z1)
                p_gin/p_ghn: lists of 2 psum tiles [128, B]
                hT_old_halves: list of 2 SBUF f32 APs [128, B]
                Returns new hT tile [128, 2*B] (mmdt)."""
                hT_new = spool.tile([128, 2 * B], mmdt, tag="hT")
                r = wpool.tile([128, 2 * B], f32, tag="r")
                z = wpool.tile([128, 2 * B], f32, tag="z")
                n = wpool.tile([128, 2 * B], f32, tag="n")
                for m in range(2):
                    sl = slice(m * B, (m + 1) * B)
                    nc.scalar.activation(r[:, sl], p_rz[m][:], AF.Sigmoid,
                                         bias=brz[:, m, :])
                    nc.scalar.activation(z[:, sl], p_rz[2 + m][:], AF.Sigmoid,
                                         bias=brz[:, 2 + m, :])
                    tmp = wpool.tile([128, B], f32, tag="tmp")
                    # tmp = (ghn + bhn) * r
                    nc.vector.scalar_tensor_tensor(
                        tmp[:], p_ghn[m][:], bhn[:, m, :], r[:, sl],
                        op0=ALU.add, op1=ALU.mult)
                    c = wpool.tile([128, B], f32, tag="c")
                    # c = (gin + bin) + tmp
                    nc.vector.scalar_tensor_tensor(
                        c[:], p_gin[m][:], bin_[:, m, :], tmp[:],
                        op0=ALU.add, op1=ALU.add)
                    nc.scalar.activation(n[:, sl], c[:], AF.Tanh)
                    hmn = wpool.tile([128, B], f32, tag="hmn")
                    nc.vector.tensor_sub(hmn[:], hT_old_halves[m], n[:, sl])
                    e = wpool.tile([128, B], f32, tag="e")
                    nc.vector.tensor_mul(e[:], z[:, sl], hmn[:])
                    nc.vector.tensor_add(hT_new[:, sl], n[:, sl], e[:])
                return hT_new

            # ================= step 0 =================
            h0bm = spool.tile([128, NBT, DIN], f32, tag="h0bm")
            nc.sync.dma_start(
                h0bm[:], h0_d.ap().rearrange("(bt p) d -> p bt d", p=128))
            e0 = load_enc(0)

            # scores0 partial [64, 64] = sum_bt h0bm[bt].T @ e0[bt]
            psc0 = pmm.tile([DIN, DIN], f32, tag="mm")
            for bt in range(NBT):
                nc.tensor.matmul(psc0[:], h0bm[:, bt, :], e0[:, bt, :],
                                 start=(bt == 0), stop=(bt == NBT - 1))
            sc0 = wpool.tile([DIN, DIN], f32, tag="sc0")
            nc.vector.tensor_copy(sc0[:], psc0[:])
            red0 = allreduce(sc0, DIN, DIN)
            w0, rinv0 = softmax_weights(red0, DIN, 1)

            # w0T [64, 64]
            pw0 = pmm.tile([DIN, DIN], f32, tag="mm")
            nc.tensor.transpose(pw0[:], w0[:], ident[0:DIN, 0:DIN])
            w0t = wpool.tile([DIN, DIN], mmdt, tag="w0t")
            nc.vector.tensor_copy(w0t[:], pw0[:])

            e0T = make_encT(e0)

            # h0T [64, 512] (psum)
            ph0 = pmm.tile([DIN, B], f32, tag="mm")
            for bt in range(NBT):
                nc.tensor.transpose(ph0[:, bt * 128:(bt + 1) * 128],
                                    h0bm[:, bt, :], ident[:])
            # cat0 [128, 512]: rows 0:64 = ctx0T (scaled), rows 64:128 = h0T
            cat0 = wpool.tile([128, B], mmdt, tag="cat0")
            nc.scalar.copy(cat0[DIN:128, :], ph0[:])

            pctx0 = pmm.tile([DIN, B], f32, tag="mm")
            nc.tensor.matmul(pctx0[:], w0t[:], e0T[:], start=True, stop=True)
            nc.vector.tensor_scalar_mul(cat0[0:DIN, :], pctx0[:],
                                        rinv0[:, 0:1])

            # att0T [64, 512] = tanh(Wa1T.T @ cat0 + ba1)
            patt0 = pmm.tile([DIN, B], f32, tag="mm")
            nc.tensor.matmul(patt0[:], wa1t[:], cat0[:], start=True, stop=True)
            att0T = wpool.tile([DIN, B], mmdt, tag="att0T")
            nc.scalar.activation(att0T[:], patt0[:], AF.Tanh, bias=ba1[:])

            # hrep [128, 512] = [h0T; h0T]  (both halves from psum ph0)
            hrep = wpool.tile([128, B], mmdt, tag="hrep")
            nc.scalar.copy(hrep[0:DIN, :], ph0[:])
            nc.scalar.copy(hrep[DIN:128, :], ph0[:])

            # GRU1 gates
            p_rz = []
            for m in range(4):
                pg = prz.tile([128, B], f32, tag="rz")
                nc.tensor.matmul(pg[:], wih1t[:, m * 128:(m + 1) * 128],
                                 att0T[:], start=True, stop=False)
                nc.tensor.matmul(pg[:], whh1et[:, m * 128:(m + 1) * 128],
                                 hrep[0:DIN, :], start=False, stop=True)
                p_rz.append(pg)
            p_gin, p_ghn = [], []
            for m in range(2):
                pg = pmm.tile([128, B], f32, tag="mm")
                nc.tensor.matmul(
                    pg[:], wih1t[:, 512 + m * 128: 512 + (m + 1) * 128],
                    att0T[:], start=True, stop=True)
                p_gin.append(pg)
            for m in range(2):
                pg = pghn.tile([128, B], f32, tag="ghn")
                nc.tensor.matmul(
                    pg[:], whh1et[:, 512 + m * 128: 512 + (m + 1) * 128],
                    hrep[0:DIN, :], start=True, stop=True)
                p_ghn.append(pg)

            hrep_f32 = asf32(hrep[:])
            hT = gru_elementwise(p_rz, p_gin, p_ghn,
                                 [hrep_f32, hrep_f32], b1rz, b1in, b1hn)
            hbm = make_hbm(hT)
            outproj(hT, 0)

            # ================= steps 1..n_steps-1 =================
            for t in range(1, n_steps):
                e_bm = load_enc(t)

                # partial scores [128, 2*64]: m-tile m at cols m*64..
                psc = pmm.tile([128, 2 * DIN], f32, tag="mm")
                for m in range(2):
                    for bt in range(NBT):
                        nc.tensor.matmul(
                            psc[:, m * DIN:(m + 1) * DIN],
                            hbm[:].rearrange("p (bt k) -> p bt k", bt=NBT)
                               [:, bt, m * 128:(m + 1) * 128],
                            e_bm[:, bt, :],
                            start=(bt == 0), stop=(bt == NBT - 1))
                sc = wpool.tile([128, 2 * DIN], f32, tag="sc")
                nc.vector.tensor_copy(sc[:], psc[:])

                red = allreduce(sc, 128, 2 * DIN)

                eT = make_encT(e_bm)

                # hT-only matmuls fill the AllReduce window: ghn + the Whh
                # half of the r/z gate accumulations (start the psum groups).
                p_ghn = []
                for m in range(2):
                    pg = pghn.tile([128, B], f32, tag="ghn")
                    for j in range(2):
                        nc.tensor.matmul(
                            pg[:], whh2t[:, j, 512 + m * 128: 512 + (m + 1) * 128],
                            hT[:, j * B:(j + 1) * B],
                            start=(j == 0), stop=(j == 1))
                    p_ghn.append(pg)
                p_rz = []
                for m in range(4):
                    pg = prz.tile([128, B], f32, tag="rz")
                    for j in range(2):
                        nc.tensor.matmul(
                            pg[:], whh2t[:, j, m * 128:(m + 1) * 128],
                            hT[:, j * B:(j + 1) * B],
                            start=(j == 0), stop=False)
                    p_rz.append(pg)

                w_sb, rinv = softmax_weights(red, 128, 2)

                # wT [64, 256]
                pw = pmm.tile([DIN, DH], f32, tag="mm")
                for m in range(2):
                    nc.tensor.transpose(pw[:, m * 128:(m + 1) * 128],
                                        w_sb[:, m * DIN:(m + 1) * DIN],
                                        ident[:])
                wt = wpool.tile([DIN, DH], mmdt, tag="wt")
                nc.vector.tensor_copy(wt[:], pw[:])

                # ctxT [128, 2*512] scaled by rinv
                ctxT = wpool.tile([128, 2 * B], mmdt, tag="ctxT")
                for m in range(2):
                    pc = pmm.tile([128, B], f32, tag="mm")
                    nc.tensor.matmul(pc[:], wt[:, m * 128:(m + 1) * 128],
                                     eT[:], start=True, stop=True)
                    nc.vector.tensor_scalar_mul(
                        ctxT[:, m * B:(m + 1) * B], pc[:], rinv[:, m:m + 1])

                # attT = tanh(Wa2T.T @ [ctxT; hT] + ba2)
                attT = wpool.tile([128, 2 * B], mmdt, tag="attT")
                for m in range(2):
                    pa = pmm.tile([128, B], f32, tag="mm")
                    for j in range(4):
                        rhs = (ctxT[:, j * B:(j + 1) * B] if j < 2
                               else hT[:, (j - 2) * B:(j - 1) * B])
                        nc.tensor.matmul(
                            pa[:], wa2t[:, j, m * 128:(m + 1) * 128],
                            rhs, start=(j == 0), stop=(j == 3))
                    nc.scalar.activation(attT[:, m * B:(m + 1) * B], pa[:],
                                         AF.Tanh, bias=ba2[:, m, :])

                # finish r/z gates with the Wih half (att-dependent)
                for m in range(4):
                    pg = p_rz[m]
                    for j in range(2):
                        nc.tensor.matmul(
                            pg[:], wih2t[:, j, m * 128:(m + 1) * 128],
                            attT[:, j * B:(j + 1) * B],
                            start=False, stop=(j == 1))
                p_gin = []
                for m in range(2):
                    pg = pmm.tile([128, B], f32, tag="mm")
                    for j in range(2):
                        nc.tensor.matmul(
                            pg[:], wih2t[:, j, 512 + m * 128: 512 + (m + 1) * 128],
                            attT[:, j * B:(j + 1) * B],
                            start=(j == 0), stop=(j == 1))
                    p_gin.append(pg)

                hT_old_halves = [asf32(hT[:, 0:B]), asf32(hT[:, B:2 * B])]
                hT = gru_elementwise(p_rz, p_gin, p_ghn, hT_old_halves,
                                     b2rz, b2in, b2hn)
                if t < n_steps - 1:
                    hbm = make_hbm(hT)
                outproj(hT, t)

    nc.compile()
    return nc


def _get_nc(n_steps=NS, mm_f32r=True):
    key = (n_steps, mm_f32r)
    if key not in _CACHE:
        _CACHE[key] = _build(n_steps, mm_f32r=mm_f32r)
    return _CACHE[key]


def _prep_weights(inputs):
    g = {k: np.ascontiguousarray(np.asarray(v, dtype=np.float32))
         for k, v in inputs.items()}
    Whh1 = g["Whh1"]
    whh1e = Whh1[:, 0:64] + Whh1[:, 64:128] + Whh1[:, 128:192] + Whh1[:, 192:256]
    wd = {
        "wa1t": g["Wa1"].T,
        "ba1c": g["ba1"][:, None],
        "wa2t": g["Wa2"].T,
        "ba2c": g["ba2"][:, None],
        "wih1t": g["Wih1"].T,
        "whh1et": whh1e.T,
        "b1rz": (g["bih1"] + g["bhh1"])[:512, None],
        "b1in": g["bih1"][512:, None],
        "b1hn": g["bhh1"][512:, None],
        "wih2t": g["Wih2"].T,
        "whh2t": g["Whh2"].T,
        "b2rz": (g["bih2"] + g["bhh2"])[:512, None],
        "b2in": g["bih2"][512:, None],
        "b2hn": g["bhh2"][512:, None],
        "woutt": g["Wout"].T,
        "boutr": np.tile(g["bout"], NBT)[None, :],
    }
    return {k: np.ascontiguousarray(v, dtype=np.float32) for k, v in wd.items()}


def run(inputs, n_steps=NS, mm_f32r=True, trace=False):
    from concourse import bass_utils
    nc = _get_nc(n_steps, mm_f32r)
    fcn = np.ascontiguousarray(np.asarray(inputs["fcn_latent"], np.float32))
    enc = np.ascontiguousarray(np.asarray(inputs["encoder_outputs"], np.float32))
    wd = _prep_weights(inputs)
    in_maps = []
    for c in range(NCORES):
        m = dict(wd)
        m["h0"] = np.ascontiguousarray(fcn[c * B:(c + 1) * B])
        m["enc"] = np.ascontiguousarray(enc[c * B:(c + 1) * B])
        in_maps.append(m)
    res = bass_utils.run_bass_kernel_spmd(
        nc, in_maps, core_ids=list(range(NCORES)), trace=trace)
    out = np.concatenate([res.results[c]["out"] for c in range(NCORES)], axis=0)
    return out[:, :n_steps, :], res


def kernel(**inputs) -> np.ndarray:
    out, _ = run(inputs, n_steps=NS, mm_f32r=True)
    return out
